# revision 1
# baseline (speedup 1.0000x reference)
"""Trainium2 Bass kernel for nn_Metric_35545149342437 (RelationNet-style few-shot metric).

Sharding: data-parallel over the 8 episodes (one per NeuronCore). Conv-stack
BatchNorm uses batch statistics over ALL episodes' images, so per-layer channel
sum/sumsq partials are AllReduced across the 8 cores (4 tiny collectives).
Everything else (all-pairs g-MLP, f-MLP, squash margin loss) is per-episode.

Self-contained: hardcodes all shapes; host packs weights into the on-chip
layouts (block-diagonal pair-packed conv weights, split MLP weights).
"""
import numpy as np

import concourse.bacc as bacc
import concourse.bass as bass
import concourse.mybir as mybir
from concourse import tile
from concourse.bass_utils import run_bass_kernel_spmd

F32 = mybir.dt.float32
F32R = mybir.dt.float32r
AF = mybir.ActivationFunctionType
ALU = mybir.AluOpType
AX = mybir.AxisListType

B, N_WAY, Q, IMG = 8, 5, 15, 84
NIMG = N_WAY + Q          # 20 images per episode/core
NPAIR = NIMG // 2         # 10 pairs; pair p = images (2p, 2p+1)
CF = 64
D2 = 9                    # 3x3 pixels after avgpool

# per-layer geometry: (in_w, out_rows, out_w, pooled_rows, pooled_w)
# L1: conv 84->82 valid (im2col, compact 82x82), pool -> 41x41
# L2: in 41x41, out rows 0..38 width 41 (valid cols 0..38), pool -> 19x19
# L3: in 19x19, out rows 0..16 width 19 (valid cols 0..16)
# L4: in 17x19 valid 17x17, out rows 0..14 width 19 (valid 15x15)
EPS = 1e-5

# BN group counts: support group = 40 images, query = 120 (over all 8 cores)
PIX = {1: 41 * 41, 2: 19 * 19, 3: 17 * 17, 4: 15 * 15}
CNT_S = {l: 40 * PIX[l] for l in PIX}
CNT_Q = {l: 120 * PIX[l] for l in PIX}


# ---------------------------------------------------------------- host packing
def _pack_weights(inp):
    """Pack all weights/consts into device layouts. Returns dict[str, np.ndarray]."""
    out = {}
    cw1 = np.asarray(inp["cw1"], np.float32)  # (64,3,3,3) (O,C,KH,KW)
    # w1blk (128,128): rows t*3+c -> cols 0:64 (img A), rows 64+t*3+c -> cols 64:128
    w1blk = np.zeros((128, 3 * 128), np.float32)
    for di in range(3):
        w1s = np.zeros((9, 64), np.float32)
        for dj in range(3):
            for c in range(3):
                w1s[dj * 3 + c] = cw1[:, c, di, dj]
        w1blk[0:9, di * 128 : di * 128 + 64] = w1s
        w1blk[64:73, di * 128 + 64 : di * 128 + 128] = w1s
    out["w1blk"] = w1blk

    for l, name in [(2, "cw2"), (3, "cw3"), (4, "cw4")]:
        cw = np.asarray(inp[name], np.float32)  # (64,64,3,3)
        blk = np.zeros((128, 9 * 128), np.float32)
        for di in range(3):
            for dj in range(3):
                t = di * 3 + dj
                wt = cw[:, :, di, dj].T  # (C_in, O)
                blk[0:64, t * 128 : t * 128 + 64] = wt
                blk[64:128, t * 128 + 64 : t * 128 + 128] = wt
        out[f"w{l}blk"] = blk

    # bn params stacked [g;g],[b;b]: (128, 8) col l*2 = g_l, l*2+1 = b_l
    bnp = np.zeros((128, 8), np.float32)
    for i, l in enumerate([1, 2, 3, 4]):
        g = np.asarray(inp[f"bg{l}"], np.float32)
        b = np.asarray(inp[f"bb{l}"], np.float32)
        bnp[0:64, i * 2] = g
        bnp[64:128, i * 2] = g
        bnp[0:64, i * 2 + 1] = b
        bnp[64:128, i * 2 + 1] = b
    out["bnp"] = bnp

    # inverse-count tiles for BN mean/var, per layer, combos [s|s, s|q, q|q]: (128, 12)
    invc = np.zeros((128, 12), np.float32)
    for i, l in enumerate([1, 2, 3, 4]):
        cs, cq = 1.0 / CNT_S[l], 1.0 / CNT_Q[l]
        invc[0:64, i * 3 + 0] = cs
        invc[64:128, i * 3 + 0] = cs
        invc[0:64, i * 3 + 1] = cs
        invc[64:128, i * 3 + 1] = cq
        invc[0:64, i * 3 + 2] = cq
        invc[64:128, i * 3 + 2] = cq
    out["invc"] = invc

    # g-MLP layer 1 split: gW1 (132,256): rows 0:66 = Ws (support), 66:132 = Wq
    gW1 = np.asarray(inp["gW1"], np.float32)
    gb1 = np.asarray(inp["gb1"], np.float32)
    ii, jj = np.meshgrid(np.arange(3), np.arange(3), indexing="ij")
    coord = (np.stack([ii, jj]).astype(np.float32) / 3.0).reshape(2, 9)  # (2,9)
    out["gwsA"] = (gW1[0:64] / 25.0).copy()      # (64,256) stationary K=64
    out["gwqB"] = (gW1[66:130] / 25.0).copy()    # (64,256)
    cA = coord.T @ gW1[64:66]                     # (9,256)
    cB = coord.T @ gW1[130:132]                   # (9,256)
    abase = np.zeros((128, 18), np.float32)       # col mh*9+p: cA[p, mh*128+row] + gb1
    qbase = np.zeros((128, 18), np.float32)
    for mh in range(2):
        abase[:, mh * 9 : mh * 9 + 9] = (cA[:, mh * 128 : mh * 128 + 128] + gb1[mh * 128 : mh * 128 + 128]).T
        qbase[:, mh * 9 : mh * 9 + 9] = cB[:, mh * 128 : mh * 128 + 128].T
    out["abase"] = abase
    out["qbase"] = qbase

    # gW2/3/4: (128, 512): col kh*256 + m
    for name in ["gW2", "gW3", "gW4", "fW1", "fW2"]:
        W = np.asarray(inp[name], np.float32)  # (256,256)
        t = np.zeros((128, 512), np.float32)
        t[:, 0:256] = W[0:128]
        t[:, 256:512] = W[128:256]
        out[name.lower() + "t"] = t
    for name in ["gb2", "gb3", "gb4", "fb1", "fb2"]:
        v = np.asarray(inp[name], np.float32)
        t = np.zeros((128, 2), np.float32)
        t[:, 0] = v[0:128]
        t[:, 1] = v[128:256]
        out[name.lower() + "t"] = t
    fW3 = np.asarray(inp["fW3"], np.float32)  # (256,128)
    t = np.zeros((128, 256), np.float32)
    t[:, 0:128] = fW3[0:128]
    t[:, 128:256] = fW3[128:256]
    out["fw3t"] = t
    fb3 = np.asarray(inp["fb3"], np.float32)
    out["fb3t"] = fb3.reshape(128, 1).copy()
    out["fw4t"] = np.asarray(inp["fW4"], np.float32).copy()  # (128,64)
    misc = np.zeros((128, 2), np.float32)
    misc[0:64, 0] = np.asarray(inp["fb4"], np.float32)
    misc[0:64, 1] = 1.0
    out["miscb"] = misc
    out["zeros"] = np.zeros((128, 3696), np.float32)
    return out


def _per_core_inputs(inp, b):
    sx = np.asarray(inp["support_x"], np.float32)[b]  # (5,3,84,84)
    qx = np.asarray(inp["query_x"], np.float32)[b]    # (15,3,84,84)
    imgs = np.concatenate([sx, qx], 0).reshape(NIMG, 3, IMG * IMG).copy()
    sy = np.asarray(inp["support_y"])[b]  # (5,)
    qy = np.asarray(inp["query_y"])[b]    # (15,)
    ap = (sy[None, :] == qy[:, None]).astype(np.float32)  # (q,s)
    apan = np.zeros((1, 150), np.float32)
    # col layout: s*15 + q
    for s in range(5):
        for q in range(15):
            apan[0, s * 15 + q] = ap[q, s]
            apan[0, 75 + s * 15 + q] = 1.0 - ap[q, s]
    return {"imgs": imgs, "apan": apan}


# ---------------------------------------------------------------- kernel build

def _apv(base, off, dims):
    """View into base AP: copy partition dim, add free dims, extra element offset."""
    return bass.AP(tensor=base.tensor, offset=base.offset + off,
                   ap=[list(base.ap[0])] + [list(d) for d in dims])

def build_kernel(debug=False):
    nc = bacc.Bacc("TRN2", target_bir_lowering=False, debug=False, num_devices=8)

    din = {}
    def dram_in(name, shape):
        din[name] = nc.dram_tensor(name, list(shape), F32, kind="ExternalInput")
        return din[name]

    imgs = dram_in("imgs", (NIMG, 3, IMG * IMG))
    apan = dram_in("apan", (1, 150))
    w1blk = dram_in("w1blk", (128, 384))
    w2blk = dram_in("w2blk", (128, 9 * 128))
    w3blk = dram_in("w3blk", (128, 9 * 128))
    w4blk = dram_in("w4blk", (128, 9 * 128))
    bnp = dram_in("bnp", (128, 8))
    invc = dram_in("invc", (128, 12))
    gwsA = dram_in("gwsA", (64, 256))
    gwqB = dram_in("gwqB", (64, 256))
    abase = dram_in("abase", (128, 18))
    qbase = dram_in("qbase", (128, 18))
    gw2t = dram_in("gw2t", (128, 512))
    gw3t = dram_in("gw3t", (128, 512))
    gw4t = dram_in("gw4t", (128, 512))
    gb2t = dram_in("gb2t", (128, 2))
    gb3t = dram_in("gb3t", (128, 2))
    gb4t = dram_in("gb4t", (128, 2))
    fw1t = dram_in("fw1t", (128, 512))
    fw2t = dram_in("fw2t", (128, 512))
    fb1t = dram_in("fb1t", (128, 2))
    fb2t = dram_in("fb2t", (128, 2))
    fw3t = dram_in("fw3t", (128, 256))
    fb3t = dram_in("fb3t", (128, 1))
    fw4t = dram_in("fw4t", (128, 64))
    miscb = dram_in("miscb", (128, 2))
    zeros = dram_in("zeros", (128, 3696))

    loss_out = nc.dram_tensor("loss", [1, 75], F32, kind="ExternalOutput")
    dbg = {}
    if debug:
        for name, shape in [
            ("d_p1n", (128, NPAIR * 1681)),
            ("d_p2n", (128, NPAIR * 361)),
            ("d_l3n", (128, NPAIR * 323)),
            ("d_l4n", (128, NPAIR * 285)),
            ("d_f64", (64, 180)),
            ("d_af", (128, 90)),
            ("d_bf", (128, 270)),
            ("d_xf", (128, 150)),
            ("d_score", (1, 150)),
        ]:
            dbg[name] = nc.dram_tensor(name, list(shape), F32, kind="ExternalOutput")

    with tile.TileContext(nc) as tc:
        with (
            tc.tile_pool(name="psum", bufs=4, space="PSUM") as psum,
            tc.tile_pool(name="dram", bufs=8, space="DRAM") as dram,
            tc.tile_pool(name="persist", bufs=1) as pp,
        ):
            # ---------------- conv phase pool
            cpool_cm = tc.tile_pool(name="convp", bufs=1)
            cp = cpool_cm.__enter__()
            w1t = cp.tile([128, 384], F32R)
            nc.scalar.dma_start(w1t[:], w1blk[:].bitcast(F32R))
            bnpt = cp.tile([128, 8], F32)
            nc.scalar.dma_start(bnpt[:], bnp[:])
            invct = cp.tile([128, 12], F32)
            nc.scalar.dma_start(invct[:], invc[:])

            # persistent activations
            p1 = [cp.tile([128, 1772], F32R, tag=f"p1_{p}", name=f"p1_{p}") for p in range(NPAIR)]
            p2 = [cp.tile([128, 368], F32R, tag=f"p2_{p}", name=f"p2_{p}") for p in range(NPAIR)]
            l3 = [cp.tile([128, 328], F32R, tag=f"l3_{p}", name=f"l3_{p}") for p in range(NPAIR)]
            l4 = [cp.tile([128, 285], F32, tag=f"l4_{p}", name=f"l4_{p}") for p in range(NPAIR)]
            feats = pp.tile([64, 184], F32R)   # (ch, img*9 + p)

            # ---------------- BN stats helper ----------------
            def bn_allreduce_and_scales(layer_i, stats, stats2):
                """stats (128,20): col 2p sums / 2p+1 sumsq per pair (half=img parity).
                Returns (scale, shift) (128,3) combo tiles [s|s, s|q, q|q]."""
                # fold odd-img halves down to partitions 0:64
                nc.sync.dma_start(stats2[0:64, :], stats[64:128, :])
                G = cp.tile([64, 4], F32, tag="G")
                tmp = cp.tile([64, 4], F32, tag="Gtmp")
                # even imgs: sup = imgs 0,2,4 (pairs 0..2 even half) cols 0,2,4
                #            qry = imgs 6..18 even: cols 6,8,..,18 (7)
                # odd imgs (stats2): sup = imgs 1,3: cols 0,2 ; qry = imgs 5..19 odd: cols 4..18 (8)
                for k, (off_e, n_e, off_o, n_o) in enumerate(
                    [(0, 3, 0, 2), (1, 3, 1, 2), (6, 7, 4, 8), (7, 7, 5, 8)]
                ):
                    ev = stats[0:64, off_e : off_e + 2 * (n_e - 1) + 1 : 2]
                    ov = stats2[0:64, off_o : off_o + 2 * (n_o - 1) + 1 : 2]
                    nc.vector.tensor_reduce(tmp[:, k : k + 1], ev, axis=AX.X, op=ALU.add)
                    nc.vector.tensor_reduce(G[:, k : k + 1], ov, axis=AX.X, op=ALU.add)
                nc.vector.tensor_tensor(G[:], G[:], tmp[:], ALU.add)
                bin_ = dram.tile([64, 4], F32)
                bout = dram.tile([64, 4], F32)
                nc.sync.dma_start(bin_[:], G[:])
                nc.gpsimd.collective_compute(
                    "AllReduce", ALU.add, ins=[bin_.opt()], outs=[bout.opt()],
                    replica_groups=[list(range(8))],
                )
                # build stacked combo stats T (128,6): cols c*2 sum, c*2+1 ss
                T = cp.tile([128, 6], F32, tag="bnT")
                nc.sync.dma_start(T[0:64, 0:2], bout[:, 0:2])
                nc.sync.dma_start(T[64:128, 0:2], bout[:, 0:2])
                nc.sync.dma_start(T[0:64, 2:4], bout[:, 0:2])
                nc.sync.dma_start(T[64:128, 2:4], bout[:, 2:4])
                nc.sync.dma_start(T[0:64, 4:6], bout[:, 2:4])
                nc.sync.dma_start(T[64:128, 4:6], bout[:, 2:4])
                ic = invct[:, (layer_i - 1) * 3 : (layer_i - 1) * 3 + 3]
                sums = T[:, 0:6:2]
                sss = T[:, 1:6:2]
                m = cp.tile([128, 3], F32, tag="bn_m")
                v = cp.tile([128, 3], F32, tag="bn_v")
                scale = cp.tile([128, 3], F32, tag=f"bn_scale{layer_i}")
                shift = cp.tile([128, 3], F32, tag=f"bn_shift{layer_i}")
                nc.vector.tensor_tensor(m[:], sums, ic, ALU.mult)
                nc.vector.tensor_tensor(v[:], sss, ic, ALU.mult)
                msq = cp.tile([128, 3], F32, tag="bn_msq")
                nc.vector.tensor_tensor(msq[:], m[:], m[:], ALU.mult)
                nc.vector.tensor_tensor(v[:], v[:], msq[:], ALU.subtract)
                nc.vector.tensor_scalar(v[:], v[:], EPS, None, ALU.add)
                nc.scalar.sqrt(v[:], v[:])
                nc.vector.reciprocal(v[:], v[:])
                g_b = bnpt[:, (layer_i - 1) * 2 : (layer_i - 1) * 2 + 1].broadcast_to((128, 3))
                b_b = bnpt[:, (layer_i - 1) * 2 + 1 : (layer_i - 1) * 2 + 2].broadcast_to((128, 3))
                nc.vector.tensor_tensor(scale[:], v[:], g_b, ALU.mult)
                nc.vector.tensor_tensor(msq[:], m[:], scale[:], ALU.mult)
                nc.vector.tensor_tensor(shift[:], b_b, msq[:], ALU.subtract)
                return scale, shift

            def combo_col(p):
                return 0 if p < 2 else (1 if p == 2 else 2)

            # ================ conv1 + pool1 ================
            stats1 = cp.tile([128, 20], F32, tag="stats")
            stats2_1 = cp.tile([128, 20], F32, tag="stats2")
            imcA = cp.tile([128, 44 * 84], F32R)   # src rows 0..43 (out rows 0..41)
            imcB = cp.tile([128, 42 * 84], F32R)   # src rows 42..83 (out rows 42..81)
            nc.sync.dma_start(feats[:, 180:184], zeros[:][0:64, :4].bitcast(F32R))
            nc.scalar.dma_start(imcA[9:64, :], zeros[:][9:64, : 44 * 84].bitcast(F32R))
            nc.sync.dma_start(imcB[9:64, :], zeros[:][9:64, : 42 * 84].bitcast(F32R))
            nc.gpsimd.dma_start(imcA[73:128, :], zeros[:][73:128, : 44 * 84].bitcast(F32R))
            nc.scalar.dma_start(imcB[73:128, :], zeros[:][73:128, : 42 * 84].bitcast(F32R))
            nc.scalar.dma_start(imcB[0:9, 42 * 84 - 2 :], zeros[:][0:9, :2].bitcast(F32R))
            nc.scalar.dma_start(imcB[64:73, 42 * 84 - 2 :], zeros[:][0:9, :2].bitcast(F32R))
            for p in range(NPAIR):
                nc.scalar.dma_start(p1[p][:, 1681:1772], zeros[:][:, :91].bitcast(F32R))
                nc.scalar.dma_start(p2[p][:, 361:368], zeros[:][:, :7].bitcast(F32R))
            sq_scr = cp.tile([128, 1681], F32, tag="sq_scr")

            for p in range(NPAIR):
                if p == 5:
                    # warm-up collective: absorbs cross-core launch skew during conv1
                    wub_in = dram.tile([1, 8], F32)
                    wub_out = dram.tile([1, 8], F32)
                    wu_s = pp.tile([1, 8], F32)
                    nc.sync.dma_start(wu_s[:], zeros[:][0:1, 0:8])
                    nc.sync.dma_start(wub_in[:], wu_s[:])
                    nc.gpsimd.collective_compute(
                        "AllReduce", ALU.add, ins=[wub_in.opt()], outs=[wub_out.opt()],
                        replica_groups=[list(range(8))],
                    )
                for half, (r0, nr, imc) in enumerate([(0, 42, imcA), (42, 40, imcB)]):
                    # ONE DMA per (img, half): 9 partitions (dj*3+c), contiguous rows
                    # r0..r0+nr+1 (halo +2 for the di taps; di applied as free offsets).
                    for hb, base in [(0, 0), (1, 64)]:
                        img = 2 * p + hb
                        L = (nr + 2) * 84 - (2 if r0 + nr + 2 >= 84 else 0)
                        dst = imc[base : base + 9, 0:L]
                        iap = imgs[:]
                        srcv = bass.AP(
                            tensor=iap.tensor,
                            offset=iap.offset + img * 3 * 7056 + r0 * 84,
                            ap=[[1, 3], [7056, 3], [1, L]],
                        ).bitcast(F32R)
                        nc.gpsimd.dma_start(dst, srcv)
                    # matmul chunks of 6 rows (pool-pair aligned)
                    nrow_left = nr
                    c0 = 0
                    while nrow_left > 0:
                        cr = min(6, nrow_left)
                        n = cr * 84
                        acc = psum.tile([128, 512], F32, tag="ps", name="ps")
                        for di in range(3):
                            nc.tensor.matmul(
                                acc[:, :n], w1t[0:128, di * 128 : (di + 1) * 128],
                                imc[0:128, c0 + di * 84 : c0 + di * 84 + n],
                                start=(di == 0), stop=(di == 2), skip_group_check=True,
                            )
                        # fused 2x2 maxpool: one reduce over (row-pair, col-pair)
                        prow = (r0 + c0 // 84) // 2
                        out_ap = p1[p][:, prow * 41 : (prow + cr // 2) * 41].rearrange(
                            "p (a b) -> p a b", a=cr // 2
                        )
                        nc.vector.tensor_reduce(
                            out_ap,
                            _apv(acc[:, :n], 0, [[168, cr // 2], [2, 41], [84, 2], [1, 2]]),
                            axis=AX.XY, op=ALU.max)
                        c0 += n
                        nrow_left -= cr
                # stats for this pair
                nc.vector.tensor_reduce(stats1[:, 2 * p : 2 * p + 1], p1[p][:, :1681].bitcast(F32), axis=AX.X, op=ALU.add)
                nc.scalar.activation(sq_scr[:, :1681], p1[p][:, :1681].bitcast(F32), AF.Square,
                                     accum_out=stats1[:, 2 * p + 1 : 2 * p + 2])

            wblk = {}
            for l, wsrc in [(2, w2blk), (3, w3blk), (4, w4blk)]:
                wblk[l] = cp.tile([128, 9 * 128], F32R, tag=f"wblk{l}", name=f"wblk{l}")
                nc.scalar.dma_start(wblk[l][:], wsrc[:].bitcast(F32R))
            # tail-phase weights, loaded up-front (overlap with conv phases)
            gwsA_t = pp.tile([64, 256], F32R)
            gwqB_t = pp.tile([64, 256], F32R)
            nc.scalar.dma_start(gwsA_t[:], gwsA[:].bitcast(F32R))
            nc.scalar.dma_start(gwqB_t[:], gwqB[:].bitcast(F32R))
            abase_t = pp.tile([128, 18], F32)
            qbase_t = pp.tile([128, 18], F32)
            nc.scalar.dma_start(abase_t[:], abase[:])
            nc.scalar.dma_start(qbase_t[:], qbase[:])
            gwt = {}
            gbt = {}
            for i, (w, b) in enumerate([(gw2t, gb2t), (gw3t, gb3t), (gw4t, gb4t)]):
                gwt[i] = pp.tile([128, 512], F32R, tag=f"gwt{i}", name=f"gwt{i}")
                nc.scalar.dma_start(gwt[i][:], w[:].bitcast(F32R))
                gbt[i] = pp.tile([128, 2], F32, tag=f"gbt{i}", name=f"gbt{i}")
                nc.scalar.dma_start(gbt[i][:], b[:])
            fw1 = pp.tile([128, 512], F32R)
            fw2 = pp.tile([128, 512], F32R)
            fw3 = pp.tile([128, 256], F32R)
            fw4 = pp.tile([128, 64], F32R)
            nc.scalar.dma_start(fw1[:], fw1t[:].bitcast(F32R))
            nc.scalar.dma_start(fw2[:], fw2t[:].bitcast(F32R))
            nc.scalar.dma_start(fw3[:], fw3t[:].bitcast(F32R))
            nc.scalar.dma_start(fw4[:], fw4t[:].bitcast(F32R))
            fb1 = pp.tile([128, 2], F32)
            fb2 = pp.tile([128, 2], F32)
            fb3 = pp.tile([128, 1], F32)
            misct = pp.tile([128, 2], F32R)
            nc.scalar.dma_start(fb1[:], fb1t[:])
            nc.scalar.dma_start(fb2[:], fb2t[:])
            nc.scalar.dma_start(fb3[:], fb3t[:])
            nc.scalar.dma_start(misct[:], miscb[:].bitcast(F32R))
            apan_t = pp.tile([1, 150], F32)
            nc.scalar.dma_start(apan_t[:], apan[:])

            sc1, sh1 = bn_allreduce_and_scales(1, stats1, stats2_1)
            p1n = []
            for p in range(NPAIR):
                c = combo_col(p)
                nc.scalar.activation(
                    p1[p][:, :1681], p1[p][:, :1681].bitcast(F32), AF.Relu,
                    bias=sh1[:, c : c + 1], scale=sc1[:, c : c + 1],
                )
                p1n.append(p1[p][:])
                if debug:
                    nc.sync.dma_start(dbg["d_p1n"][:][:, p * 1681 : (p + 1) * 1681], p1[p][:, :1681].bitcast(F32))

            # ================ conv2 + pool2 ================
            stats1b = cp.tile([128, 20], F32, tag="stats_b")
            stats2b = cp.tile([128, 20], F32, tag="stats2_b")
            for p in range(NPAIR):
                for ci, (r0, nr) in enumerate([(0, 12), (12, 12), (24, 12), (36, 4)]):
                    n_out = nr * 41
                    acc = psum.tile([128, 512], F32, tag="ps", name="ps")
                    for di in range(3):
                        for dj in range(3):
                            t = di * 3 + dj
                            off = di * 41 + dj
                            n = n_out
                            nc.tensor.matmul(
                                acc[:, :n], wblk[2][:, t * 128 : (t + 1) * 128],
                                p1n[p][:, r0 * 41 + off : r0 * 41 + off + n],
                                start=(t == 0), stop=(t == 8), skip_group_check=True,
                            )
                    nu = 12 if nr == 12 else 2  # rows used by pooling
                    prow = r0 // 2
                    out_ap = p2[p][:, prow * 19 : (prow + nu // 2) * 19].rearrange(
                        "p (a b) -> p a b", a=nu // 2
                    )
                    nc.vector.tensor_reduce(
                        out_ap,
                        _apv(acc[:, :n_out], 0, [[82, nu // 2], [2, 19], [41, 2], [1, 2]]),
                        axis=AX.XY, op=ALU.max)
                nc.vector.tensor_reduce(stats1b[:, 2 * p : 2 * p + 1], p2[p][:, :361].bitcast(F32), axis=AX.X, op=ALU.add)
                nc.scalar.activation(sq_scr[:, :361], p2[p][:, :361].bitcast(F32), AF.Square,
                                     accum_out=stats1b[:, 2 * p + 1 : 2 * p + 2])

            sc2, sh2 = bn_allreduce_and_scales(2, stats1b, stats2b)
            p2n = []
            for p in range(NPAIR):
                c = combo_col(p)
                nc.scalar.activation(
                    p2[p][:, :361], p2[p][:, :361].bitcast(F32), AF.Relu,
                    bias=sh2[:, c : c + 1], scale=sc2[:, c : c + 1],
                )
                p2n.append(p2[p][:])
                if debug:
                    nc.sync.dma_start(dbg["d_p2n"][:][:, p * 361 : (p + 1) * 361], p2[p][:, :361].bitcast(F32))

            # ================ conv3 (no pool) ================
            stats1c = cp.tile([128, 20], F32, tag="stats_c")
            stats2c = cp.tile([128, 20], F32, tag="stats2_c")
            for p in range(NPAIR):
                nc.scalar.dma_start(l3[p][:], zeros[:][:, :328].bitcast(F32R))
                acc = psum.tile([128, 512], F32, tag="ps", name="ps")
                for di in range(3):
                    for dj in range(3):
                        t = di * 3 + dj
                        off = di * 19 + dj
                        n = 324
                        nc.tensor.matmul(
                            acc[:, :n], wblk[3][:, t * 128 : (t + 1) * 128],
                            p2n[p][:, off : off + n],
                            start=(t == 0), stop=(t == 8), skip_group_check=True,
                        )
                vps = acc[:, :323].rearrange("p (a b) -> p a b", a=17)[:, :, 0:17]
                vl3 = l3[p][:, :323].rearrange("p (a b) -> p a b", a=17)[:, :, 0:17]
                nc.scalar.activation(vl3, vps, AF.Copy,
                                     accum_out=stats1c[:, 2 * p : 2 * p + 1])
                nc.scalar.activation(sq_scr[:, :289].rearrange("p (a b) -> p a b", a=17),
                                     vl3.bitcast(F32), AF.Square,
                                     accum_out=stats1c[:, 2 * p + 1 : 2 * p + 2])

            sc3, sh3 = bn_allreduce_and_scales(3, stats1c, stats2c)
            l3n = []
            for p in range(NPAIR):
                c = combo_col(p)
                vl3 = l3[p][:, :323].rearrange("p (a b) -> p a b", a=17)[:, :, 0:17]
                nc.scalar.activation(
                    vl3, vl3.bitcast(F32), AF.Relu,
                    bias=sh3[:, c : c + 1], scale=sc3[:, c : c + 1],
                )
                l3n.append(l3[p][:])
                if debug:
                    nc.sync.dma_start(dbg["d_l3n"][:][:, p * 323 : (p + 1) * 323], l3[p][:, :323].bitcast(F32))

            # ================ conv4 (no pool) ================
            stats1d = cp.tile([128, 20], F32, tag="stats_d")
            stats2d = cp.tile([128, 20], F32, tag="stats2_d")
            fall = cp.tile([128, 90], F32, tag="fall")
            for p in range(NPAIR):
                nc.gpsimd.memset(l4[p][:], 0.0)
                acc = psum.tile([128, 512], F32, tag="ps", name="ps")
                for di in range(3):
                    for dj in range(3):
                        t = di * 3 + dj
                        off = di * 19 + dj
                        n = 288
                        nc.tensor.matmul(
                            acc[:, :n], wblk[4][:, t * 128 : (t + 1) * 128],
                            l3n[p][:, off : off + n],
                            start=(t == 0), stop=(t == 8), skip_group_check=True,
                        )
                vps = acc[:, :285].rearrange("p (a b) -> p a b", a=15)[:, :, 0:15]
                vl4 = l4[p][:].rearrange("p (a b) -> p a b", a=15)[:, :, 0:15]
                nc.scalar.activation(vl4, vps, AF.Copy,
                                     accum_out=stats1d[:, 2 * p : 2 * p + 1])
                nc.scalar.activation(sq_scr[:, :225].rearrange("p (a b) -> p a b", a=15),
                                     vl4, AF.Square,
                                     accum_out=stats1d[:, 2 * p + 1 : 2 * p + 2])

            sc4, sh4 = bn_allreduce_and_scales(4, stats1d, stats2d)
            for p in range(NPAIR):
                c = combo_col(p)
                vl4 = l4[p][:].rearrange("p (a b) -> p a b", a=15)[:, :, 0:15]
                nc.scalar.activation(
                    vl4, vl4, AF.Relu,
                    bias=sh4[:, c : c + 1], scale=sc4[:, c : c + 1],
                )
                if debug:
                    nc.sync.dma_start(dbg["d_l4n"][:][:, p * 285 : (p + 1) * 285], l4[p][:])
                # avgpool 5x5 (sum; /25 folded into gwsA/gwqB) -> fall
                inv = _apv(l4[p][:], 0, [[95, 3], [5, 3], [19, 5], [1, 5]])
                nc.vector.tensor_reduce(fall[:, p * 9 : (p + 1) * 9], inv, axis=AX.XY, op=ALU.add)
            # batched feats assembly: evens from fall[0:64], odds from fall[64:128]
            for hb in range(2):
                dstv = _apv(feats[:, hb * 9 : hb * 9 + 9], 0, [[18, 10], [1, 9]])
                srcv = _apv(fall[hb * 64 : hb * 64 + 64, :], 0, [[9, 10], [1, 9]])
                nc.sync.dma_start(dstv, srcv.bitcast(F32R))
            if debug:
                nc.sync.dma_start(dbg["d_f64"][:], feats[:, :180].bitcast(F32))

            cpool_cm.__exit__(None, None, None)

            # ================ g-MLP ================
            tpool_cm = tc.tile_pool(name="tailp", bufs=1)
            tp = tpool_cm.__enter__()

            A_f = [tp.tile([128, 45], F32, tag=f"A_f{k}", name=f"A_f{k}") for k in range(2)]
            B_f = [tp.tile([128, 135], F32, tag=f"B_f{k}", name=f"B_f{k}") for k in range(2)]
            for mh in range(2):
                accA = psum.tile([128, 48], F32, tag="ps", name="ps")
                nc.tensor.matmul(accA[:], gwsA_t[:, mh * 128 : (mh + 1) * 128],
                                 feats[:, 0:48], start=True, stop=True)
                bav = abase_t[:, mh * 9 : (mh + 1) * 9].unsqueeze(1).broadcast_to((128, 5, 9))
                nc.vector.tensor_tensor(
                    A_f[mh][:].rearrange("p (a b) -> p a b", a=5),
                    accA[:, :45].rearrange("p (a b) -> p a b", a=5), bav, ALU.add)
                accB = psum.tile([128, 136], F32, tag="ps", name="ps")
                nc.tensor.matmul(accB[:], gwqB_t[:, mh * 128 : (mh + 1) * 128],
                                 feats[:, 45:181], start=True, stop=True)
                qbv = qbase_t[:, mh * 9 : (mh + 1) * 9].unsqueeze(1).broadcast_to((128, 15, 9))
                nc.vector.tensor_tensor(
                    B_f[mh][:].rearrange("p (a b) -> p a b", a=15),
                    accB[:, :135].rearrange("p (a b) -> p a b", a=15), qbv, ALU.add)
            if debug:
                for mh in range(2):
                    nc.sync.dma_start(dbg["d_af"][:][:, mh * 45 : (mh + 1) * 45], A_f[mh][:])
                    nc.sync.dma_start(dbg["d_bf"][:][:, mh * 135 : (mh + 1) * 135], B_f[mh][:])

            NH = 6076  # 45 * 135 + 1 pad (f32r matmul N % 4 == 0)
            with tc.tile_pool(name="hpool", bufs=4) as hpool:
                h_in = [hpool.tile([128, NH], F32R, tag="h", name="h") for _ in range(2)]
                for kh in range(2):
                    nc.sync.dma_start(h_in[kh][:, 6075:6076], zeros[:][:, :1].bitcast(F32R))
                for kh in range(2):
                    for sp in range(45):
                        nc.vector.tensor_scalar(
                            h_in[kh][:, sp * 135 : (sp + 1) * 135], B_f[kh][:],
                            A_f[kh][:, sp : sp + 1], 0.0, ALU.add, ALU.max,
                        )
                # layers 2..4
                chunks = [(i * 512, 512) for i in range(11)] + [(5632, 444)]
                for li in range(3):
                    h_out = [hpool.tile([128, NH], F32R, tag="h", name="h") for _ in range(2)]
                    for mh in range(2):
                        for c0, n in chunks:
                            acc = psum.tile([128, 512], F32, tag="ps", name="ps")
                            nc.tensor.matmul(acc[:, :n], gwt[li][:, mh * 128 : mh * 128 + 128],
                                             h_in[0][:, c0 : c0 + n], start=True, stop=False)
                            nc.tensor.matmul(acc[:, :n], gwt[li][:, 256 + mh * 128 : 256 + mh * 128 + 128],
                                             h_in[1][:, c0 : c0 + n], start=False, stop=True)
                            nc.scalar.activation(
                                h_out[mh][:, c0 : c0 + n], acc[:, :n], AF.Relu,
                                bias=gbt[li][:, mh : mh + 1],
                            )
                    h_in = h_out

                # x_f = sum over (p1, p2): cols (s*9+p1)*135 + q*9+p2
                xf = [tp.tile([128, 76], F32R, tag=f"xf{k}", name=f"xf{k}") for k in range(2)]
                for k_ in range(2):
                    nc.sync.dma_start(xf[k_][:, 75:76], zeros[:][:, :1].bitcast(F32R))
                R = tp.tile([128, 675], F32, tag="Racc")
                for kh in range(2):
                    hv = h_in[kh][:].bitcast(F32)
                    for s in range(5):
                        rblk = R[:, s * 135 : (s + 1) * 135]
                        nc.vector.tensor_copy(rblk, hv[:, s * 1215 : s * 1215 + 135])
                        for p1 in range(1, 9):
                            o = s * 1215 + p1 * 135
                            nc.vector.tensor_tensor(rblk, rblk, hv[:, o : o + 135], ALU.add)
                    # stage B: over p2 (stride 9): view [[135,5],[9,15]] offset p2
                    xacc = tp.tile([128, 75], F32, tag="xacc")
                    def pview(p2_):
                        return _apv(R[:], p2_, [[135, 5], [9, 15]])
                    xv = xacc[:].rearrange("p (a b) -> p a b", a=5)
                    nc.vector.tensor_copy(xv, pview(0))
                    for p2_ in range(1, 8):
                        nc.vector.tensor_tensor(xv, xv, pview(p2_), ALU.add)
                    nc.vector.tensor_tensor(
                        xf[kh][:, :75].rearrange("p (a b) -> p a b", a=5), xv, pview(8), ALU.add)
                if debug:
                    for kh in range(2):
                        nc.sync.dma_start(dbg["d_xf"][:][:, kh * 75 : (kh + 1) * 75],
                                          xf[kh][:, :75].bitcast(F32))

            # ================ f-MLP + score + loss ================

            y_in = xf
            for li, (w, bias, mhs) in enumerate([(fw1, fb1, 2), (fw2, fb2, 2)]):
                y_out = [tp.tile([128, 76], F32R, tag=f"y{li}_{k}", name=f"y{li}_{k}") for k in range(mhs)]
                for mh in range(mhs):
                    acc = psum.tile([128, 76], F32, tag="ps", name="ps")
                    nc.tensor.matmul(acc[:], w[:, mh * 128 : mh * 128 + 128],
                                     y_in[0][:], start=True, stop=False)
                    nc.tensor.matmul(acc[:], w[:, 256 + mh * 128 : 256 + mh * 128 + 128],
                                     y_in[1][:], start=False, stop=True)
                    nc.scalar.activation(y_out[mh][:], acc[:], AF.Relu,
                                         bias=bias[:, mh : mh + 1])
                y_in = y_out
            # fW3: 256 -> 128
            y3 = tp.tile([128, 76], F32R, tag="y3")
            acc = psum.tile([128, 76], F32, tag="ps", name="ps")
            nc.tensor.matmul(acc[:], fw3[:, 0:128], y_in[0][:], start=True, stop=False)
            nc.tensor.matmul(acc[:], fw3[:, 128:256], y_in[1][:], start=False, stop=True)
            nc.scalar.activation(y3[:], acc[:], AF.Relu, bias=fb3[:, 0:1])
            # fW4: 128 -> 64 ; then (o + fb4)^2
            acc4 = psum.tile([64, 76], F32, tag="ps", name="ps")
            nc.tensor.matmul(acc4[:], fw4[:], y3[:], start=True, stop=True)
            osq = tp.tile([64, 76], F32R, tag="osq")
            nc.scalar.activation(osq[:], acc4[:], AF.Square,
                                 bias=misct[0:64, 0:1].bitcast(F32))
            # score^2 = colsum(osq) via ones matmul; squash+margin-loss done on host
            acc_sc = psum.tile([1, 76], F32, tag="ps", name="ps")
            nc.tensor.matmul(acc_sc[:], misct[0:64, 1:2], osq[:], start=True, stop=True)
            sc2t = tp.tile([1, 76], F32, tag="sc2")
            nc.vector.tensor_copy(sc2t[:], acc_sc[:])
            nc.sync.dma_start(loss_out[:], sc2t[:, :75])
            tpool_cm.__exit__(None, None, None)

    nc.compile()
    return nc


# ---------------------------------------------------------------- entry point
_CACHE = {}


def finish_loss(results, inputs):
    """Host epilogue: squash + margin loss from per-core score^2 (75 flops/core)."""
    sy = np.asarray(inputs["support_y"])
    qy = np.asarray(inputs["query_y"])
    total = np.float32(0.0)
    for b in range(B):
        sc2 = np.asarray(results[b]["loss"][0], np.float32)  # (75,) col = s*15+q
        score = np.sqrt(np.maximum(sc2, 0.0)).reshape(5, 15).T  # (q, s)
        n = np.sqrt((score * score).sum(1, keepdims=True))
        score = score / n * (n * n / (1.0 + n * n))
        ap = sy[b][None, :] == qy[b][:, None]
        sap = np.sum(np.where(ap, score, 0.0), axis=1, keepdims=True)
        total += np.float32(np.sum(np.maximum(score - sap + 0.2, 0.0) * (~ap)))
    return np.array(total, dtype=np.float32)


def kernel(**inputs) -> np.ndarray:
    if "nc" not in _CACHE:
        _CACHE["nc"] = build_kernel(debug=False)
    nc = _CACHE["nc"]
    packed = _pack_weights(inputs)
    in_maps = []
    for b in range(B):
        m = dict(packed)
        m.update(_per_core_inputs(inputs, b))
        in_maps.append(m)
    res = run_bass_kernel_spmd(nc, in_maps, core_ids=list(range(8)))
    return finish_loss(res.results, inputs)



# revision 17
# speedup vs baseline: 1.1276x; 1.1276x over previous
"""Trainium2 Bass kernel for nn_Metric_35545149342437 (RelationNet-style few-shot metric).

Sharding: data-parallel over the 8 episodes (one per NeuronCore). Conv-stack
BatchNorm uses batch statistics over ALL episodes' images; per-layer channel
sum/sumsq partials are AllReduced across the 8 cores, split into a support-
group and a query-group collective per layer so each one's latency hides
behind compute (support stats complete early — after pair 2 — and query-group
results are only needed from pair 2 of the next conv layer onward).

All matmuls run in bf16 (fp32 PSUM accumulation); elementwise work is spread
across scalar/vector/gpsimd engines. conv1 uses a 27-tap im2col (K=54 with
two images packed) so each output chunk is a single matmul.
"""
import numpy as np
import ml_dtypes

import concourse.bacc as bacc
import concourse.bass as bass
import concourse.mybir as mybir
from concourse import tile
from concourse.bass_utils import run_bass_kernel_spmd

F32 = mybir.dt.float32
F32R = mybir.dt.float32r
BF16 = mybir.dt.bfloat16
AF = mybir.ActivationFunctionType
ALU = mybir.AluOpType
AX = mybir.AxisListType

B, N_WAY, Q, IMG = 8, 5, 15, 84
NIMG = N_WAY + Q          # 20 images per episode/core
NPAIR = NIMG // 2         # 10 pairs; pair p = images (2p, 2p+1)
CF = 64
EPS = 1e-5
IMG2 = IMG * IMG          # 7056
IMGSTRIDE = 3 * IMG2 + 8  # padded per-image stride (elements) for im2col tail reads

PIX = {1: 41 * 41, 2: 19 * 19, 3: 17 * 17, 4: 15 * 15}
CNT_S = {l: 40 * PIX[l] for l in PIX}
CNT_Q = {l: 120 * PIX[l] for l in PIX}

NPBF = ml_dtypes.bfloat16


# ---------------------------------------------------------------- host packing
def _pack_weights(inp):
    out = {}
    cw1 = np.asarray(inp["cw1"], np.float32)  # (64,3,3,3) (O,C,KH,KW)
    # w1t (64,128): row p=c*9+di*3+dj -> cols 0:64 (img A); row 27+p -> cols 64:128
    w1t = np.zeros((64, 128), np.float32)
    for di in range(3):
        for dj in range(3):
            for c in range(3):
                p = c * 9 + di * 3 + dj
                w1t[p, 0:64] = cw1[:, c, di, dj]
                w1t[27 + p, 64:128] = cw1[:, c, di, dj]
    out["w1t"] = w1t.astype(NPBF)

    for l, name in [(2, "cw2"), (3, "cw3"), (4, "cw4")]:
        cw = np.asarray(inp[name], np.float32)  # (64,64,3,3)
        blk = np.zeros((128, 9 * 128), np.float32)
        for di in range(3):
            for dj in range(3):
                t = di * 3 + dj
                wt = cw[:, :, di, dj].T  # (C_in, O)
                blk[0:64, t * 128 : t * 128 + 64] = wt
                blk[64:128, t * 128 + 64 : t * 128 + 128] = wt
        out[f"w{l}blk"] = blk.astype(NPBF)

    # bn params stacked [g;g],[b;b]: (128, 8) col l*2 = g_l, l*2+1 = b_l
    bnp = np.zeros((128, 8), np.float32)
    for i, l in enumerate([1, 2, 3, 4]):
        g = np.asarray(inp[f"bg{l}"], np.float32)
        b = np.asarray(inp[f"bb{l}"], np.float32)
        bnp[0:64, i * 2] = g
        bnp[64:128, i * 2] = g
        bnp[0:64, i * 2 + 1] = b
        bnp[64:128, i * 2 + 1] = b
    out["bnp"] = bnp

    # inverse-count tiles for BN mean/var, per layer, combos [s|s, s|q, q|q]: (128, 12)
    invc = np.zeros((128, 12), np.float32)
    for i, l in enumerate([1, 2, 3, 4]):
        cs, cq = 1.0 / CNT_S[l], 1.0 / CNT_Q[l]
        invc[0:64, i * 3 + 0] = cs
        invc[64:128, i * 3 + 0] = cs
        invc[0:64, i * 3 + 1] = cs
        invc[64:128, i * 3 + 1] = cq
        invc[0:64, i * 3 + 2] = cq
        invc[64:128, i * 3 + 2] = cq
    out["invc"] = invc

    # g-MLP layer 1 split: gW1 (132,256): rows 0:66 = Ws (support), 66:132 = Wq
    gW1 = np.asarray(inp["gW1"], np.float32)
    gb1 = np.asarray(inp["gb1"], np.float32)
    ii, jj = np.meshgrid(np.arange(3), np.arange(3), indexing="ij")
    coord = (np.stack([ii, jj]).astype(np.float32) / 3.0).reshape(2, 9)  # (2,9)
    out["gwsA"] = (gW1[0:64] / 25.0).astype(NPBF)      # (64,256) stationary K=64
    out["gwqB"] = (gW1[66:130] / 25.0).astype(NPBF)    # (64,256)
    cA = coord.T @ gW1[64:66]                           # (9,256)
    cB = coord.T @ gW1[130:132]                         # (9,256)
    abase = np.zeros((128, 18), np.float32)             # col mh*9+p: cA[p, mh*128+row] + gb1
    qbase = np.zeros((128, 18), np.float32)
    for mh in range(2):
        abase[:, mh * 9 : mh * 9 + 9] = (cA[:, mh * 128 : mh * 128 + 128] + gb1[mh * 128 : mh * 128 + 128]).T
        qbase[:, mh * 9 : mh * 9 + 9] = cB[:, mh * 128 : mh * 128 + 128].T
    out["abase"] = abase
    out["qbase"] = qbase

    # gW2/3/4, fW1/2: (128, 512): col kh*256 + m
    for name in ["gW2", "gW3", "gW4", "fW1", "fW2"]:
        W = np.asarray(inp[name], np.float32)  # (256,256)
        t = np.zeros((128, 512), np.float32)
        t[:, 0:256] = W[0:128]
        t[:, 256:512] = W[128:256]
        out[name.lower() + "t"] = t.astype(NPBF)
    for name in ["gb2", "gb3", "gb4", "fb1", "fb2"]:
        v = np.asarray(inp[name], np.float32)
        t = np.zeros((128, 2), np.float32)
        t[:, 0] = v[0:128]
        t[:, 1] = v[128:256]
        out[name.lower() + "t"] = t
    fW3 = np.asarray(inp["fW3"], np.float32)  # (256,128)
    t = np.zeros((128, 256), np.float32)
    t[:, 0:128] = fW3[0:128]
    t[:, 128:256] = fW3[128:256]
    out["fw3t"] = t.astype(NPBF)
    fb3 = np.asarray(inp["fb3"], np.float32)
    out["fb3t"] = fb3.reshape(128, 1).copy()
    out["fw4t"] = np.asarray(inp["fW4"], np.float32).astype(NPBF)  # (128,64)
    misc = np.zeros((128, 2), np.float32)
    misc[0:64, 0] = np.asarray(inp["fb4"], np.float32)
    misc[0:64, 1] = 1.0
    out["miscb"] = misc
    out["zbf"] = np.zeros((128, 512), NPBF)
    return out


def _per_core_inputs(inp, b):
    sx = np.asarray(inp["support_x"], np.float32)[b]  # (5,3,84,84)
    qx = np.asarray(inp["query_x"], np.float32)[b]    # (15,3,84,84)
    raw = np.concatenate([sx, qx], 0).reshape(NIMG, 3 * IMG2)
    imgs = np.zeros((NIMG, IMGSTRIDE), NPBF)
    imgs[:, : 3 * IMG2] = raw.astype(NPBF)
    return {"imgs": imgs}


# ---------------------------------------------------------------- kernel build
def _apv(base, off, dims):
    """View into base AP: copy partition dim, add free dims, extra element offset."""
    return bass.AP(tensor=base.tensor, offset=base.offset + off,
                   ap=[list(base.ap[0])] + [list(d) for d in dims])


def build_kernel(debug=False):
    nc = bacc.Bacc("TRN2", target_bir_lowering=False, debug=False, num_devices=8)

    def dram_in(name, shape, dt=F32):
        return nc.dram_tensor(name, list(shape), dt, kind="ExternalInput")

    imgs = dram_in("imgs", (NIMG, IMGSTRIDE), BF16)
    w1blk = dram_in("w1t", (64, 128), BF16)
    w2blk = dram_in("w2blk", (128, 9 * 128), BF16)
    w3blk = dram_in("w3blk", (128, 9 * 128), BF16)
    w4blk = dram_in("w4blk", (128, 9 * 128), BF16)
    bnp = dram_in("bnp", (128, 8))
    invc = dram_in("invc", (128, 12))
    gwsA = dram_in("gwsA", (64, 256), BF16)
    gwqB = dram_in("gwqB", (64, 256), BF16)
    abase = dram_in("abase", (128, 18))
    qbase = dram_in("qbase", (128, 18))
    gw2t = dram_in("gw2t", (128, 512), BF16)
    gw3t = dram_in("gw3t", (128, 512), BF16)
    gw4t = dram_in("gw4t", (128, 512), BF16)
    gb2t = dram_in("gb2t", (128, 2))
    gb3t = dram_in("gb3t", (128, 2))
    gb4t = dram_in("gb4t", (128, 2))
    fw1t = dram_in("fw1t", (128, 512), BF16)
    fw2t = dram_in("fw2t", (128, 512), BF16)
    fb1t = dram_in("fb1t", (128, 2))
    fb2t = dram_in("fb2t", (128, 2))
    fw3t = dram_in("fw3t", (128, 256), BF16)
    fb3t = dram_in("fb3t", (128, 1))
    fw4t = dram_in("fw4t", (128, 64), BF16)
    miscb = dram_in("miscb", (128, 2))
    zbf = dram_in("zbf", (128, 512), BF16)

    loss_out = nc.dram_tensor("loss", [1, 75], F32, kind="ExternalOutput")
    dbg = {}
    if debug:
        for name, shape, dt in [
            ("d_p1n", (128, NPAIR * 1681), BF16),
            ("d_p2n", (128, NPAIR * 361), BF16),
            ("d_l3n", (128, NPAIR * 323), BF16),
            ("d_l4n", (128, NPAIR * 285), BF16),
            ("d_f64", (64, 184), BF16),
            ("d_xf", (128, 152), F32),
            ("d_G", (64, 16), F32),
        ]:
            dbg[name] = nc.dram_tensor(name, list(shape), dt, kind="ExternalOutput")

    with tile.TileContext(nc) as tc:
        with (
            tc.tile_pool(name="psum", bufs=4, space="PSUM") as psum,
            tc.tile_pool(name="dram", bufs=16, space="DRAM") as dram,
            tc.tile_pool(name="persist", bufs=1) as pp,
        ):
            cpool_cm = tc.tile_pool(name="convp", bufs=1)
            cp = cpool_cm.__enter__()
            w1t = cp.tile([64, 128], BF16)
            nc.scalar.dma_start(w1t[:], w1blk[:])
            bnpt = cp.tile([128, 8], F32)
            nc.scalar.dma_start(bnpt[:], bnp[:])
            invct = cp.tile([128, 12], F32)
            nc.scalar.dma_start(invct[:], invc[:])
            wblk = {}
            for l, wsrc in [(2, w2blk), (3, w3blk), (4, w4blk)]:
                wblk[l] = cp.tile([128, 9 * 128], BF16, tag=f"wblk{l}", name=f"wblk{l}")
                nc.scalar.dma_start(wblk[l][:], wsrc[:])

            # persistent activations (bf16)
            p1 = [cp.tile([128, 1772], BF16, tag=f"p1_{p}", name=f"p1_{p}") for p in range(NPAIR)]
            p2 = [cp.tile([128, 368], BF16, tag=f"p2_{p}", name=f"p2_{p}") for p in range(NPAIR)]
            l3 = [cp.tile([128, 328], BF16, tag=f"l3_{p}", name=f"l3_{p}") for p in range(NPAIR)]
            l4 = [cp.tile([128, 288], BF16, tag=f"l4_{p}", name=f"l4_{p}") for p in range(NPAIR)]
            feats = pp.tile([64, 184], BF16)
            nc.sync.dma_start(feats[:, 180:184], zbf[:][0:64, 0:4])
            for _p in range(NPAIR):
                nc.sync.dma_start(p1[_p][:, 1681:1772], zbf[:][:, 0:91])
                nc.sync.dma_start(p2[_p][:, 361:368], zbf[:][:, 0:7])

            # per-layer stats (col 2p = sum, 2p+1 = sumsq; partition halves = img parity)
            stats = {l: cp.tile([128, 20], F32, tag=f"st{l}", name=f"st{l}") for l in [1, 2, 3, 4]}
            stats2 = {l: cp.tile([64, 20], F32, tag=f"st2{l}", name=f"st2{l}") for l in [1, 2, 3, 4]}
            sq_scr = cp.tile([128, 1681], BF16, tag="sq_scr")
            bout = {}   # (layer, grp) -> dram AllReduce output tile

            # ---------------- BN helpers ----------------
            def launch_layer_allreduce(layer):
                """One (64,4) AllReduce per layer: [sup_sum, sup_ss, qry_sum, qry_ss]."""
                st, st2 = stats[layer], stats2[layer]
                nc.sync.dma_start(st2[:, 0:20], st[64:128, 0:20])
                G = cp.tile([64, 4], F32, tag="G", name="G")
                tmp = cp.tile([64, 4], F32, tag="Gtmp", name="Gtmp")
                for k, (off_e, n_e, off_o, n_o) in enumerate(
                    [(0, 3, 0, 2), (1, 3, 1, 2), (6, 7, 4, 8), (7, 7, 5, 8)]
                ):
                    ev = st[0:64, off_e : off_e + 2 * (n_e - 1) + 1 : 2]
                    ov = st2[0:64, off_o : off_o + 2 * (n_o - 1) + 1 : 2]
                    nc.vector.tensor_reduce(tmp[:, k : k + 1], ev, axis=AX.X, op=ALU.add)
                    nc.vector.tensor_reduce(G[:, k : k + 1], ov, axis=AX.X, op=ALU.add)
                nc.vector.tensor_tensor(G[:], G[:], tmp[:], ALU.add)
                bin_ = dram.tile([64, 4], F32, name="arin")
                bo = dram.tile([64, 4], F32, name="arout")
                nc.sync.dma_start(bin_[:], G[:])
                nc.gpsimd.collective_compute(
                    "AllReduce", ALU.add, ins=[bin_.opt()], outs=[bo.opt()],
                    replica_groups=[list(range(8))],
                )
                bout[layer] = bo
                if debug:
                    nc.sync.dma_start(dbg["d_G"][:][:, (layer - 1) * 4 : (layer - 1) * 4 + 4], bo[:])

            scT = {}
            shT = {}
            bnT = {}

            def compute_scales(layer, part):
                """part 0: combo col 0 (needs support AR); part 1: cols 1,2 (needs query)."""
                if layer not in scT:
                    scT[layer] = cp.tile([128, 3], F32, tag=f"sc{layer}", name=f"sc{layer}")
                    shT[layer] = cp.tile([128, 3], F32, tag=f"sh{layer}", name=f"sh{layer}")
                    bnT[layer] = cp.tile([128, 6], F32, tag=f"bnT{layer}", name=f"bnT{layer}")
                T = bnT[layer]
                bo = bout[layer]
                if part == 0:
                    nc.sync.dma_start(T[0:64, 0:2], bo[:, 0:2])
                    nc.sync.dma_start(T[64:128, 0:2], bo[:, 0:2])
                    cs, ce = 0, 1
                else:
                    nc.sync.dma_start(T[0:64, 2:4], bo[:, 0:2])
                    nc.sync.dma_start(T[64:128, 2:4], bo[:, 2:4])
                    nc.sync.dma_start(T[0:64, 4:6], bo[:, 2:4])
                    nc.sync.dma_start(T[64:128, 4:6], bo[:, 2:4])
                    cs, ce = 1, 3
                n = ce - cs
                ic = invct[:, (layer - 1) * 3 + cs : (layer - 1) * 3 + ce]
                sums = T[:, 2 * cs : 2 * ce : 2]
                sss = T[:, 2 * cs + 1 : 2 * ce : 2]
                m = cp.tile([128, 3], F32, tag="bn_m", name="bn_m")[:, 0:n]
                v = cp.tile([128, 3], F32, tag="bn_v", name="bn_v")[:, 0:n]
                msq = cp.tile([128, 3], F32, tag="bn_msq", name="bn_msq")[:, 0:n]
                nc.vector.tensor_tensor(m, sums, ic, ALU.mult)
                nc.vector.tensor_tensor(v, sss, ic, ALU.mult)
                nc.vector.tensor_tensor(msq, m, m, ALU.mult)
                nc.vector.tensor_tensor(v, v, msq, ALU.subtract)
                nc.vector.tensor_scalar(v, v, EPS, None, ALU.add)
                nc.scalar.sqrt(v, v)
                nc.vector.reciprocal(v, v)
                g_b = bnpt[:, (layer - 1) * 2 : (layer - 1) * 2 + 1].broadcast_to((128, n))
                b_b = bnpt[:, (layer - 1) * 2 + 1 : (layer - 1) * 2 + 2].broadcast_to((128, n))
                sc = scT[layer][:, cs:ce]
                sh = shT[layer][:, cs:ce]
                nc.vector.tensor_tensor(sc, v, g_b, ALU.mult)
                nc.vector.tensor_tensor(msq, m, sc, ALU.mult)
                nc.vector.tensor_tensor(sh, b_b, msq, ALU.subtract)

            def combo_col(p):
                return 0 if p < 2 else (1 if p == 2 else 2)

            def relu_apply(layer, view, c, lane):
                """view: AP to normalize+relu in place. c: combo col. lane: 's'|'v'|'g'."""
                sc = scT[layer][:, c : c + 1]
                sh = shT[layer][:, c : c + 1]
                if lane == "s":
                    nc.scalar.activation(view, view, AF.Relu, bias=sh, scale=sc)
                else:
                    eng = nc.vector if lane == "v" else nc.gpsimd
                    eng.tensor_scalar(view, view, sc, sh, ALU.mult, ALU.add)
                    eng.tensor_scalar(view, view, 0.0, None, ALU.max)

            def stats_pair(layer, p, view, nelem):
                """sum via vector reduce; sumsq via scalar Square w/ accum (baseline style)."""
                st = stats[layer]
                nc.vector.tensor_reduce(st[:, 2 * p : 2 * p + 1], view, axis=AX.X, op=ALU.add)
                sq = sq_scr[:, :nelem]
                nc.scalar.activation(sq, view, AF.Square,
                                     accum_out=st[:, 2 * p + 1 : 2 * p + 2])

            # ================ conv1 + pool1 ================
            # im2col: 27 partitions per image (p = di*9+dj*3+ch), 2 images -> K=54
            imcs = [cp.tile([54, 3528], BF16, tag=f"imc{i}", name=f"imc{i}") for i in range(4)]
            scrs = cp  # lane-b scratch allocated on the fly via tags

            dma_engines = [nc.sync, nc.scalar, nc.gpsimd]

            def conv1_pair(p):
                for half, (r0, nr) in enumerate([(0, 42), (42, 40)]):
                    imc = imcs[(p * 2 + half) % 4]
                    L = nr * 84
                    for hb in range(2):
                        img = 2 * p + hb
                        iap = imgs[:]
                        for ch in range(3):
                            srcv = bass.AP(
                                tensor=iap.tensor,
                                offset=iap.offset + img * IMGSTRIDE + ch * 7056 + r0 * 84,
                                ap=[[84, 3], [1, 3], [1, L]],
                            )
                            eng = dma_engines[(p * 4 + half * 2 + hb + ch) % 3]
                            eng.dma_start(imc[hb * 27 + ch * 9 : hb * 27 + ch * 9 + 9, 0:L], srcv)
                    nrow_left = nr
                    c0 = 0
                    ci = 0
                    while nrow_left > 0:
                        cr = min(6, nrow_left)
                        n = cr * 84
                        acc = psum.tile([128, 512], F32, tag="ps", name="ps")
                        nc.tensor.matmul(
                            acc[:, :n], w1t[0:54, :], imc[0:54, c0 : c0 + n],
                            start=True, stop=True, skip_group_check=True,
                        )
                        prow = (r0 + c0 // 84) // 2
                        out_ap = p1[p][:, prow * 41 : (prow + cr // 2) * 41].rearrange(
                            "p (a b) -> p a b", a=cr // 2)
                        nc.vector.tensor_reduce(
                            out_ap,
                            _apv(acc[:, :n], 0, [[168, cr // 2], [2, 41], [84, 2], [1, 2]]),
                            axis=AX.XY, op=ALU.max)
                        c0 += n
                        nrow_left -= cr
                        ci += 1
                stats_pair(1, p, p1[p][:, :1681], 1681)

            for p in range(NPAIR):
                conv1_pair(p)
                if p == 0:
                    # warmup collective: absorbs cross-core launch skew
                    wub_in = dram.tile([1, 8], F32, name="wubi")
                    wub_out = dram.tile([1, 8], F32, name="wubo")
                    wu_s = pp.tile([1, 16], F32)
                    nc.sync.dma_start(wu_s[:, 0:8], zbf[:][0:1, 0:16].bitcast(F32))
                    nc.sync.dma_start(wub_in[:], wu_s[:, 0:8])
                    nc.gpsimd.collective_compute(
                        "AllReduce", ALU.add, ins=[wub_in.opt()], outs=[wub_out.opt()],
                        replica_groups=[list(range(8))],
                    )
            launch_layer_allreduce(1)

            # tail-phase weights, loaded up-front (overlap with conv phases)
            gwsA_t = pp.tile([64, 256], BF16)
            gwqB_t = pp.tile([64, 256], BF16)
            nc.scalar.dma_start(gwsA_t[:], gwsA[:])
            nc.scalar.dma_start(gwqB_t[:], gwqB[:])
            abase_t = pp.tile([128, 18], F32)
            qbase_t = pp.tile([128, 18], F32)
            nc.scalar.dma_start(abase_t[:], abase[:])
            nc.scalar.dma_start(qbase_t[:], qbase[:])
            gwt = {}
            gbt = {}
            for i, (w, b) in enumerate([(gw2t, gb2t), (gw3t, gb3t), (gw4t, gb4t)]):
                gwt[i] = pp.tile([128, 512], BF16, tag=f"gwt{i}", name=f"gwt{i}")
                nc.scalar.dma_start(gwt[i][:], w[:])
                gbt[i] = pp.tile([128, 2], F32, tag=f"gbt{i}", name=f"gbt{i}")
                nc.scalar.dma_start(gbt[i][:], b[:])
            fw1 = pp.tile([128, 512], BF16)
            fw2 = pp.tile([128, 512], BF16)
            fw3 = pp.tile([128, 256], BF16)
            fw4 = pp.tile([128, 64], BF16)
            nc.scalar.dma_start(fw1[:], fw1t[:])
            nc.scalar.dma_start(fw2[:], fw2t[:])
            nc.scalar.dma_start(fw3[:], fw3t[:])
            nc.scalar.dma_start(fw4[:], fw4t[:])
            fb1 = pp.tile([128, 2], F32)
            fb2 = pp.tile([128, 2], F32)
            fb3 = pp.tile([128, 1], F32)
            misct = pp.tile([128, 2], F32R)
            nc.scalar.dma_start(fb1[:], fb1t[:])
            nc.scalar.dma_start(fb2[:], fb2t[:])
            nc.scalar.dma_start(fb3[:], fb3t[:])
            nc.scalar.dma_start(misct[:], miscb[:].bitcast(F32R))

            # ================ conv2 + pool2 ================
            CHUNKS2 = [(0, 12), (12, 12), (24, 12), (36, 4)]

            def conv2_pair(p):
                for ci, (r0, nr) in enumerate(CHUNKS2):
                    n = nr * 41
                    acc = psum.tile([128, 512], F32, tag="ps", name="ps")
                    for t in range(9):
                        di, dj = t // 3, t % 3
                        off = di * 41 + dj
                        nc.tensor.matmul(
                            acc[:, :n], wblk[2][:, t * 128 : (t + 1) * 128],
                            p1[p][:, r0 * 41 + off : r0 * 41 + off + n],
                            start=(t == 0), stop=(t == 8), skip_group_check=True,
                        )
                    nu = 12 if nr == 12 else 2
                    prow = r0 // 2
                    out_ap = p2[p][:, prow * 19 : (prow + nu // 2) * 19].rearrange(
                        "p (a b) -> p a b", a=nu // 2)
                    nc.vector.tensor_reduce(
                        out_ap,
                        _apv(acc[:, :n], 0, [[82, nu // 2], [2, 19], [41, 2], [1, 2]]),
                        axis=AX.XY, op=ALU.max)
                stats_pair(2, p, p2[p][:, :361], 361)

            RELU_LANES = ["s", "v", "s", "v", "s", "v", "s", "v", "s", "v"]

            compute_scales(1, 0)
            for p in range(2):
                relu_apply(1, p1[p][:, :1681], 0, "s" if p == 0 else "v")
                if debug:
                    nc.sync.dma_start(dbg["d_p1n"][:][:, p * 1681 : (p + 1) * 1681], p1[p][:, :1681])
                conv2_pair(p)
            compute_scales(1, 1)
            for p in range(2, NPAIR):
                relu_apply(1, p1[p][:, :1681], combo_col(p), RELU_LANES[p])
                if debug:
                    nc.sync.dma_start(dbg["d_p1n"][:][:, p * 1681 : (p + 1) * 1681], p1[p][:, :1681])
            for p in range(2, NPAIR):
                conv2_pair(p)
            launch_layer_allreduce(2)

            # ================ conv3 (no pool) ================
            def conv34_pair(layer, p, src, dst, nvalid):
                if layer == 3:
                    nc.sync.dma_start(dst[:, 0:328], zbf[:][:, 0:328])
                acc = psum.tile([128, 512], F32, tag="ps", name="ps")
                for t in range(9):
                    di, dj = t // 3, t % 3
                    off = di * 19 + dj
                    n = 324 if layer == 3 else 288
                    nc.tensor.matmul(
                        acc[:, :n], wblk[layer][:, t * 128 : (t + 1) * 128],
                        src[:, off : off + n],
                        start=(t == 0), stop=(t == 8), skip_group_check=True,
                    )
                w = nvalid  # 17 or 15
                vps = acc[:, : w * 19].rearrange("p (a b) -> p a b", a=w)[:, :, 0:w]
                vdst = dst[:, : w * 19].rearrange("p (a b) -> p a b", a=w)[:, :, 0:w]
                st = stats[layer]
                nc.scalar.activation(vdst, vps, AF.Copy, accum_out=st[:, 2 * p : 2 * p + 1])
                sq = sq_scr[:, : w * w].rearrange("p (a b) -> p a b", a=w)
                nc.scalar.activation(sq, vdst, AF.Square,
                                     accum_out=st[:, 2 * p + 1 : 2 * p + 2])

            compute_scales(2, 0)
            for p in range(2):
                relu_apply(2, p2[p][:, :361], 0, "s" if p == 0 else "v")
                if debug:
                    nc.sync.dma_start(dbg["d_p2n"][:][:, p * 361 : (p + 1) * 361], p2[p][:, :361])
                conv34_pair(3, p, p2[p][:], l3[p][:], 17)
            compute_scales(2, 1)
            for p in range(2, NPAIR):
                relu_apply(2, p2[p][:, :361], combo_col(p), RELU_LANES[p])
                if debug:
                    nc.sync.dma_start(dbg["d_p2n"][:][:, p * 361 : (p + 1) * 361], p2[p][:, :361])
            for p in range(2, NPAIR):
                conv34_pair(3, p, p2[p][:], l3[p][:], 17)
            launch_layer_allreduce(3)

            # ================ conv4 (no pool) ================
            def vview(t, w):
                return t[:, : w * 19].rearrange("p (a b) -> p a b", a=w)[:, :, 0:w]

            compute_scales(3, 0)
            for p in range(2):
                relu_apply(3, vview(l3[p][:], 17), 0, "s" if p == 0 else "v")
                if debug:
                    nc.sync.dma_start(dbg["d_l3n"][:][:, p * 323 : (p + 1) * 323], l3[p][:, :323])
                conv34_pair(4, p, l3[p][:], l4[p][:], 15)
            compute_scales(3, 1)
            for p in range(2, NPAIR):
                relu_apply(3, vview(l3[p][:], 17), combo_col(p), RELU_LANES[p])
                if debug:
                    nc.sync.dma_start(dbg["d_l3n"][:][:, p * 323 : (p + 1) * 323], l3[p][:, :323])
            for p in range(2, NPAIR):
                conv34_pair(4, p, l3[p][:], l4[p][:], 15)
            launch_layer_allreduce(4)

            # ================ relu4 + avgpool ================
            fall = cp.tile([128, 90], F32, tag="fall")

            def pool4(p):
                inv = _apv(l4[p][:], 0, [[95, 3], [5, 3], [19, 5], [1, 5]])
                nc.vector.tensor_reduce(fall[:, p * 9 : (p + 1) * 9], inv, axis=AX.XY, op=ALU.add)

            compute_scales(4, 0)
            for p in range(2):
                relu_apply(4, vview(l4[p][:], 15), 0, "s" if p == 0 else "v")
                if debug:
                    nc.sync.dma_start(dbg["d_l4n"][:][:, p * 285 : (p + 1) * 285], l4[p][:, :285])
                pool4(p)
            compute_scales(4, 1)
            for p in range(2, NPAIR):
                relu_apply(4, vview(l4[p][:], 15), combo_col(p), RELU_LANES[p])
                if debug:
                    nc.sync.dma_start(dbg["d_l4n"][:][:, p * 285 : (p + 1) * 285], l4[p][:, :285])
                pool4(p)
            # convert to bf16 and assemble feats (ch, img*9 + p)
            fallb = cp.tile([128, 90], BF16, tag="fallb")
            nc.vector.tensor_copy(fallb[:], fall[:])
            for hb in range(2):
                dstv = _apv(feats[:, hb * 9 : hb * 9 + 9], 0, [[18, 10], [1, 9]])
                srcv = _apv(fallb[hb * 64 : hb * 64 + 64, :], 0, [[9, 10], [1, 9]])
                nc.sync.dma_start(dstv, srcv)
            if debug:
                nc.sync.dma_start(dbg["d_f64"][:], feats[:])

            cpool_cm.__exit__(None, None, None)

            # ================ g-MLP ================
            tpool_cm = tc.tile_pool(name="tailp", bufs=1)
            tp = tpool_cm.__enter__()

            A_f = [tp.tile([128, 45], BF16, tag=f"A_f{k}", name=f"A_f{k}") for k in range(2)]
            B_f = [tp.tile([128, 135], BF16, tag=f"B_f{k}", name=f"B_f{k}") for k in range(2)]
            for mh in range(2):
                accA = psum.tile([128, 48], F32, tag="ps", name="ps")
                nc.tensor.matmul(accA[:], gwsA_t[:, mh * 128 : (mh + 1) * 128],
                                 feats[:, 0:48], start=True, stop=True)
                bav = abase_t[:, mh * 9 : (mh + 1) * 9].unsqueeze(1).broadcast_to((128, 5, 9))
                nc.vector.tensor_tensor(
                    A_f[mh][:].rearrange("p (a b) -> p a b", a=5),
                    accA[:, :45].rearrange("p (a b) -> p a b", a=5), bav, ALU.add)
                accB = psum.tile([128, 136], F32, tag="ps", name="ps")
                nc.tensor.matmul(accB[:], gwqB_t[:, mh * 128 : (mh + 1) * 128],
                                 feats[:, 45:181], start=True, stop=True)
                qbv = qbase_t[:, mh * 9 : (mh + 1) * 9].unsqueeze(1).broadcast_to((128, 15, 9))
                nc.vector.tensor_tensor(
                    B_f[mh][:].rearrange("p (a b) -> p a b", a=15),
                    accB[:, :135].rearrange("p (a b) -> p a b", a=15), qbv, ALU.add)

            NH = 6076  # 45 * 135 + 1 pad
            SPL = 28   # h-build split point (vector: sp<SPL, gpsimd: rest)
            with tc.tile_pool(name="hpool", bufs=4) as hpool:
                h_in = [hpool.tile([128, NH], BF16, tag="h", name="h") for _ in range(2)]
                for kh in range(2):
                    # h[sp*135 + j] = relu(B[j] + A[sp]); add via broadcast TT, relu on scalar
                    bv = _apv(B_f[kh][:], 0, [[0, SPL], [1, 135]])
                    av = _apv(A_f[kh][:], 0, [[1, SPL], [0, 135]])
                    ov = _apv(h_in[kh][:], 0, [[135, SPL], [1, 135]])
                    nc.vector.tensor_tensor(ov, bv, av, ALU.add)
                    bv2 = _apv(B_f[kh][:], 0, [[0, 45 - SPL], [1, 135]])
                    av2 = _apv(A_f[kh][:], SPL, [[1, 45 - SPL], [0, 135]])
                    ov2 = _apv(h_in[kh][:], SPL * 135, [[135, 45 - SPL], [1, 135]])
                    nc.vector.tensor_tensor(ov2, bv2, av2, ALU.add)
                    nc.scalar.activation(h_in[kh][:, 0 : SPL * 135], h_in[kh][:, 0 : SPL * 135], AF.Relu)
                    nc.scalar.activation(h_in[kh][:, SPL * 135 : 6075], h_in[kh][:, SPL * 135 : 6075], AF.Relu)
                    nc.sync.dma_start(h_in[kh][:, 6075:6076], zbf[:][:, 0:1])

                chunks = [(i * 512, 512) for i in range(11)] + [(5632, 444)]
                for li in range(3):
                    h_out = [hpool.tile([128, NH], BF16, tag="h", name="h") for _ in range(2)]
                    for mh in range(2):
                        for ci, (c0, n) in enumerate(chunks):
                            acc = psum.tile([128, 512], F32, tag="ps", name="ps")
                            nc.tensor.matmul(acc[:, :n], gwt[li][:, mh * 128 : mh * 128 + 128],
                                             h_in[0][:, c0 : c0 + n], start=True, stop=False)
                            nc.tensor.matmul(acc[:, :n], gwt[li][:, 256 + mh * 128 : 256 + mh * 128 + 128],
                                             h_in[1][:, c0 : c0 + n], start=False, stop=True)
                            dst = h_out[mh][:, c0 : c0 + n]
                            if (ci + mh) % 2 == 0:
                                nc.scalar.activation(dst, acc[:, :n], AF.Relu,
                                                     bias=gbt[li][:, mh : mh + 1])
                            else:
                                nc.vector.tensor_scalar(dst, acc[:, :n], gbt[li][:, mh : mh + 1],
                                                        0.0, ALU.add, ALU.max)
                        if li == 2:
                            nc.sync.dma_start(h_out[mh][:, 6075:6076], zbf[:][:, 0:1])
                    h_in = h_out

                # x_f[(s,q)] = sum over (p1,p2) of h[(s,p1,q,p2)]
                xff = [tp.tile([128, 76], F32, tag=f"xff{k}", name=f"xff{k}") for k in range(2)]
                for kh in range(2):
                    hv = h_in[kh][:]
                    for s in range(5):
                        inv = _apv(hv, s * 1215, [[9, 15], [135, 9], [1, 9]])
                        nc.vector.tensor_reduce(
                            xff[kh][:, s * 15 : (s + 1) * 15], inv, axis=AX.XY, op=ALU.add)
                if debug:
                    for kh in range(2):
                        nc.sync.dma_start(dbg["d_xf"][:][:, kh * 76 : (kh + 1) * 76], xff[kh][:])

            xfb = [tp.tile([128, 76], BF16, tag=f"xfb{k}", name=f"xfb{k}") for k in range(2)]
            for kh in range(2):
                nc.vector.tensor_copy(xfb[kh][:, 0:75], xff[kh][:, 0:75])
                nc.sync.dma_start(xfb[kh][:, 75:76], zbf[:][:, 0:1])

            # ================ f-MLP + score + loss ================
            y_in = xfb
            for li, (w, bias) in enumerate([(fw1, fb1), (fw2, fb2)]):
                y_out = [tp.tile([128, 76], BF16, tag=f"y{li}_{k}", name=f"y{li}_{k}") for k in range(2)]
                for mh in range(2):
                    acc = psum.tile([128, 76], F32, tag="ps", name="ps")
                    nc.tensor.matmul(acc[:], w[:, mh * 128 : mh * 128 + 128],
                                     y_in[0][:], start=True, stop=False)
                    nc.tensor.matmul(acc[:], w[:, 256 + mh * 128 : 256 + mh * 128 + 128],
                                     y_in[1][:], start=False, stop=True)
                    nc.scalar.activation(y_out[mh][:], acc[:], AF.Relu,
                                         bias=bias[:, mh : mh + 1])
                y_in = y_out
            y3 = tp.tile([128, 76], BF16, tag="y3")
            acc = psum.tile([128, 76], F32, tag="ps", name="ps")
            nc.tensor.matmul(acc[:], fw3[:, 0:128], y_in[0][:], start=True, stop=False)
            nc.tensor.matmul(acc[:], fw3[:, 128:256], y_in[1][:], start=False, stop=True)
            nc.scalar.activation(y3[:], acc[:], AF.Relu, bias=fb3[:, 0:1])
            acc4 = psum.tile([64, 76], F32, tag="ps", name="ps")
            nc.tensor.matmul(acc4[:], fw4[:], y3[:], start=True, stop=True)
            osq = tp.tile([64, 76], F32R, tag="osq")
            nc.scalar.activation(osq[:], acc4[:], AF.Square,
                                 bias=misct[0:64, 0:1].bitcast(F32))
            acc_sc = psum.tile([1, 76], F32, tag="ps", name="ps")
            nc.tensor.matmul(acc_sc[:], misct[0:64, 1:2], osq[:], start=True, stop=True)
            sc2t = tp.tile([1, 76], F32, tag="sc2")
            nc.vector.tensor_copy(sc2t[:], acc_sc[:])
            nc.sync.dma_start(loss_out[:], sc2t[:, :75])
            tpool_cm.__exit__(None, None, None)

    nc.compile()
    return nc


# ---------------------------------------------------------------- entry point
_CACHE = {}


def finish_loss(results, inputs):
    """Host epilogue: squash + margin loss from per-core score^2 (75 flops/core)."""
    sy = np.asarray(inputs["support_y"])
    qy = np.asarray(inputs["query_y"])
    total = np.float32(0.0)
    for b in range(B):
        sc2 = np.asarray(results[b]["loss"][0], np.float32)  # (75,) col = s*15+q
        score = np.sqrt(np.maximum(sc2, 0.0)).reshape(5, 15).T  # (q, s)
        n = np.sqrt((score * score).sum(1, keepdims=True))
        score = score / n * (n * n / (1.0 + n * n))
        ap = sy[b][None, :] == qy[b][:, None]
        sap = np.sum(np.where(ap, score, 0.0), axis=1, keepdims=True)
        total += np.float32(np.sum(np.maximum(score - sap + 0.2, 0.0) * (~ap)))
    return np.array(total, dtype=np.float32)


def kernel(**inputs) -> np.ndarray:
    if "nc" not in _CACHE:
        _CACHE["nc"] = build_kernel(debug=False)
    nc = _CACHE["nc"]
    packed = _pack_weights(inputs)
    in_maps = []
    for b in range(B):
        m = dict(packed)
        m.update(_per_core_inputs(inputs, b))
        in_maps.append(m)
    res = run_bass_kernel_spmd(nc, in_maps, core_ids=list(range(8)))
    return finish_loss(res.results, inputs)
